# revision 35
# baseline (speedup 1.0000x reference)
"""Causal attention (B=1, H=16, S=4096, D=64, f32) on 8 trn2 NeuronCores.

Strategy (head-parallel, 2 heads per core):
  - Host pre-transposes Q, K per head to [D, S] (d-major) so the QK^T
    matmul needs no on-device transpose: S^T[k, q] = sum_d K^T[d,k] Q^T[d,q].
  - S^T layout keeps k on PSUM partitions and q on the free axis, so
    exp(S^T) -> P^T lands in SBUF exactly as the lhsT of the PV matmul:
    O^T[d, q] = sum_k V[k, d] P^T[k, q], accumulated over k-tiles in PSUM.
  - p' = exp(s - 3.25): the global shift (softmax-invariant, cancels in
    the l division) keeps p' <= ~190 so it fits fp8e4m3.  l[q] = sum_k p'
    comes free from a ones column in V.
  - Causality: k-tiles strictly below the diagonal are skipped; diagonal
    k-tiles are masked post-exp (VectorE) and column-trimmed (for
    diagonal tile t only q >= 128t can be unmasked).
  - Host epilogue: O = (O^T_unnorm[:64] / l).T per head.

exp is split between ScalarE (native ACT exp) and VectorE (Schraudolph
bit-trick: p_bits = round(A*s + B) -> int16, bitcast fp16), load-balanced
at build time.  q-block 0 (rows with <512 keys, least error averaging)
is pinned to the exact ScalarE fp16 path.

PV matmul precision/speed:
  - ScalarE chunks (j>=1) emit p' in fp8e4m3; their PV runs as ONE
    DoubleRow matmul per 2 k-tiles (fp8 V, contraction 256 virtual rows,
    2 elem/cycle moving) - half the PE time of two fp16 matmuls.
    Diagonal masking for these is a bitwise AND (0x00/0xFF bytes) on the
    int16-bitcast fp8 pairs, in place.
  - VectorE chunks emit fp16 (int8 Schraudolph can't represent the fp8
    subnormal band correctly), PV is two regular fp16 matmuls.
  - q-block 0 is all-fp16 (fp8 V quantization is too coarse for rows
    attending few keys).

QK^T matmuls run fp16, two-at-a-time in disjoint PE row groups (rows
0-63 / 64-127 hold duplicate q,k data) - the trace confirms the second
of each pair retires in ~4ns.

Pipeline: chunks of 2 k-tiles, one 2-bank PSUM score tile each (3 bufs -
PSUM's 8 banks = 3x2 score slots + 2 output accumulators, the binding
resource).  Chunks are emitted in batches of 3 aligned to the score-slot
rotation: the 3 QK pairs run back-to-back (hiding each other's
LDWEIGHTS), PVs trail by 2 batches so they never head-of-line block the
PE queue, and each batch's exps are emitted before the previous batch's
masks (exps gate the score-slot rotation; masks have 2 batches of
slack).

Warmup matmuls read a built-in const AP (no DMA dependency) and keep the
PE HAM activity monitor busy so the clock ramps toward 2.4 GHz before
real matmuls start.
"""

import os
import sys
import numpy as np

sys.path.insert(0, "/opt/trn_rl_repo")

import concourse.bass as bass
import concourse.mybir as mybir
from concourse.tile import TileContext

B, H, S, D = 1, 16, 4096, 64
N_CORES = 8
H_PER = H // N_CORES          # heads per core
QB = 512                      # q-block (matmul moving dim / PSUM bank)
KT = 128                      # k-tile (contraction tile for PV matmul)
NQB = S // QB                 # 8
NKT = S // KT                 # 32
VW = D + 1                    # V columns + ones column for the l sum
VWP = 80                      # fp8 V plane pitch (DoubleRow needs 16B-aligned)

F32 = mybir.dt.float32
F16 = mybir.dt.float16
F8 = mybir.dt.float8e4
I16 = mybir.dt.int16
BF16 = mybir.dt.bfloat16

LOG2E = 1.4426950408889634
SHIFT = 3.25                  # p' = exp(s - SHIFT); max p' ~ exp(8.44-3.25)=180
# Schraudolph exp for fp16 bit pattern: exp(0.125*s - SHIFT) ~=
# bitcast_fp16(round(A*s + B)); the -44.5 centers the relative error (~3%).
SCHRAU_A = 0.125 * LOG2E * 1024.0
SCHRAU_B = 15360.0 - 1024.0 * SHIFT * LOG2E - 44.5


def build_program() -> bass.Bass:
    dve_frac = float(os.environ.get("ATTN_DVE", "1"))
    use_fp8 = os.environ.get("ATTN_FP8", "1") != "0"
    n_warm = int(os.environ.get("ATTN_WARM", "0"))

    nc = bass.Bass()
    # qk rows 0-63 and 64-127 hold identical qT|kT data: the duplicate lets
    # two QK^T matmuls run concurrently in disjoint PE row groups
    qk_d = nc.declare_dram_parameter("qk", [H_PER, 2 * D, 2 * S], F16, isOutput=False)
    va_d = nc.declare_dram_parameter("va", [H_PER, 128, NKT * VW], F16, isOutput=False)
    va8_d = nc.declare_dram_parameter(
        "va8", [H_PER, 128, NKT // 2, 2, VWP], F8, isOutput=False
    )
    mk_d = nc.declare_dram_parameter("mk", [128, 4 * QB + 2], F16, isOutput=False)
    mk8_d = nc.declare_dram_parameter("mk8", [128, 2, 2, QB // 2], I16, isOutput=False)
    oT_d = nc.declare_dram_parameter("outT", [H_PER, VW, S], F32, isOutput=True)

    with TileContext(nc) as tc:
        with (
            tc.tile_pool(name="const", bufs=1) as cpool,
            tc.tile_pool(name="io", bufs=1) as iopool,
            tc.tile_pool(name="pt", bufs=8) as ppool,
            tc.tile_pool(name="pm", bufs=12) as pmpool,
            tc.tile_pool(name="st", bufs=3, space="PSUM") as stpool,
            tc.tile_pool(name="ot", bufs=2, space="PSUM") as otpool,
        ):
            # 0/1 masks for the 4 diagonal k-tiles of each q-block:
            # keep (1.0 / 0xFF) where qq >= kk + 128*t.
            mks = cpool.tile([128, 4 * QB + 2], F16, name="mks")
            nc.sync.dma_start(out=mks, in_=mk_d[:, :])
            dmasks = [mks[:, t * QB:(t + 1) * QB] for t in range(4)]
            # exp bias (-SHIFT) const AP: fp32 bit pattern embedded in the
            # last two fp16 mask columns (avoids a gpsimd memset + barrier)
            nc.const_aps.aps[(mybir.dt.float32, -SHIFT)] = (
                mks[:, 4 * QB:4 * QB + 2].bitcast(F32)
            )
            mk8s = cpool.tile([128, 2, 2, QB // 2], I16, name="mk8s")
            if use_fp8:
                nc.sync.dma_start(out=mk8s, in_=mk8_d[:, :, :, :])

            # Optional warmup matmuls (off by default: trace shows the
            # tiny 1-col matmuls have too little duty cycle to trip the PE
            # HAM clock-gate, so they were pure PE-queue time; the clock
            # ramps from real matmul activity either way).
            if n_warm:
                cb = nc.const_aps.aps[(mybir.dt.bfloat16, 1.0)]
                wps = otpool.tile([128, 16], F32, name="warmps", tag="otp")
                for _ in range(n_warm):
                    nc.tensor.matmul(
                        out=wps[0:1, 0:1], lhsT=cb, rhs=cb, start=True, stop=True,
                    )

            head_ctx = []
            for h in range(H_PER):
                vas = iopool.tile([128, NKT * VW], F16, name=f"vas{h}")
                vas8 = iopool.tile([128, NKT // 2, 2, VWP], F8, name=f"vas8{h}")
                qkts = iopool.tile([2 * D, 2 * S], F16, name=f"qkts{h}")
                outs = iopool.tile([VW, S], F32, name=f"outs{h}")
                # q-block 0 only needs the first 512 columns of q/k and the
                # first 4 V k-tiles: stage those first so compute starts
                # while the bulk still streams in
                if h == 0:
                    nc.sync.dma_start(out=vas[:, 0:4 * VW], in_=va_d[h][:, 0:4 * VW])
                    nc.sync.dma_start(out=qkts[:, 0:QB], in_=qk_d[h][:, 0:QB])
                    nc.sync.dma_start(
                        out=qkts[:, S:S + QB], in_=qk_d[h][:, S:S + QB]
                    )
                    nc.sync.dma_start(
                        out=vas[:, 4 * VW:], in_=va_d[h][:, 4 * VW:]
                    )
                    nc.sync.dma_start(out=qkts[:, QB:S], in_=qk_d[h][:, QB:S])
                    nc.sync.dma_start(
                        out=qkts[:, S + QB:2 * S], in_=qk_d[h][:, S + QB:2 * S]
                    )
                else:
                    nc.sync.dma_start(out=vas, in_=va_d[h])
                    # split halves onto separate DMA queues
                    nc.sync.dma_start(out=qkts[:, 0:S], in_=qk_d[h][:, 0:S])
                    nc.sync.dma_start(
                        out=qkts[:, S:2 * S], in_=qk_d[h][:, S:2 * S]
                    )
                if use_fp8:
                    nc.sync.dma_start(out=vas8, in_=va8_d[h])
                head_ctx.append((vas, vas8, qkts, outs))

            # flat chunk list over (head, q-block): 2 k-tiles per chunk.
            all_chunks = []
            for h in range(H_PER):
                for j in range(NQB):
                    n_kt = 4 * (j + 1)          # causal: k-tiles 0..4j+3
                    for k0 in range(0, n_kt, 2):
                        all_chunks.append((h, j, k0, n_kt))

            def chunk_off(j, k0):
                """Uniform column offset for the chunk (both k-tiles of a
                chunk are diagonal together); for diagonal pair (t, t+1)
                only q >= 128t can be unmasked."""
                t0 = k0 - 4 * j
                return 128 * t0 if t0 >= 0 else -1   # -1 = not diagonal

            # Build-time exp load balancing: ScalarE chunks (j>=1) go fp8
            # (DoubleRow PV); VectorE chunks go fp16 Schraudolph.  VectorE
            # also owns the diagonal masking.  q-block 0 stays exact fp16.
            exp_on_dve = {}
            copy_on_dve = {}
            load_s, load_d = 0.0, 0.0
            for idx, (h, j, k0, n_kt) in enumerate(all_chunks):
                off0 = chunk_off(j, k0)
                diag = off0 >= 0
                o = max(off0, 0)
                if diag:
                    t_s = 2 * ((QB - o) + 352) / 1.2
                    t_d = 2 * ((QB - o) + 151) / 0.96
                    # masks: fp8 chunk = one AND over both planes;
                    # fp16 chunk = one multiply per k-tile
                    m_s = ((QB - o) / 2 + 151) / 0.96 if use_fp8 else 2 * (
                        (QB - o) / 2 + 151
                    ) / 0.96
                    m_d = 2 * ((QB - o) / 2 + 151) / 0.96
                else:
                    t_s = (2 * QB + 352) / 1.2
                    t_d = (2 * QB + 151) / 0.96
                    m_s = m_d = 0.0
                if j == 0 or dve_frac == 0.0:
                    use_d = False
                else:
                    use_d = (load_d + (t_d + m_d) * dve_frac
                             < load_s + t_s + m_s - load_d * 0)
                exp_on_dve[idx] = use_d
                if use_d:
                    load_d += t_d + m_d
                else:
                    load_s += t_s
                    load_d += m_s
                if k0 + 2 == n_kt:   # q-block end: PSUM->SBUF copy
                    use_dc = load_d + 658 < load_s + 720
                    copy_on_dve[idx] = use_dc
                    if use_dc:
                        load_d += 658
                    else:
                        load_s += 720

            def is_fp8(idx):
                h, j, k0, n_kt = all_chunks[idx]
                return use_fp8 and j >= 1 and not exp_on_dve[idx]

            otp_box = {}

            def emit_mm1s(idx, chunk):
                h, j, k0, n_kt = chunk
                vas, vas8, qkts, outs = head_ctx[h]
                off0 = max(chunk_off(j, k0), 0)
                stp = stpool.tile([128, 2 * QB], F32, name="stp", tag="stp")
                for r in range(2):
                    ki = k0 + r
                    row = slice(r * D, (r + 1) * D)
                    nc.tensor.matmul(
                        out=stp[:, r * QB + off0:(r + 1) * QB],
                        lhsT=qkts[row, S + ki * KT:S + (ki + 1) * KT],
                        rhs=qkts[row, j * QB + off0:(j + 1) * QB],
                        start=True,
                        stop=True,
                    )
                if is_fp8(idx):
                    pt = ppool.tile([128, 2, QB], F8, name="pt8", tag="pt")
                    if off0 == 0:
                        nc.scalar.activation(
                            out=pt[:, :, :], in_=stp[:, 0:2 * QB],
                            func=mybir.ActivationFunctionType.Exp,
                            scale=0.125, bias=-SHIFT,
                        )
                    else:
                        for r in range(2):
                            nc.scalar.activation(
                                out=pt[:, r, off0:QB],
                                in_=stp[:, r * QB + off0:(r + 1) * QB],
                                func=mybir.ActivationFunctionType.Exp,
                                scale=0.125, bias=-SHIFT,
                            )
                    return pt
                pt = ppool.tile([128, 2 * QB], F16, name="pt", tag="pt")
                ranges = (
                    [(0, 2 * QB)] if off0 == 0
                    else [(r * QB + off0, (r + 1) * QB) for r in range(2)]
                )
                for a, b in ranges:
                    if exp_on_dve[idx]:
                        nc.vector.tensor_scalar(
                            out=pt[:, a:b].bitcast(I16),
                            in0=stp[:, a:b],
                            scalar1=SCHRAU_A,
                            scalar2=SCHRAU_B,
                            op0=mybir.AluOpType.mult,
                            op1=mybir.AluOpType.add,
                        )
                    else:
                        nc.scalar.activation(
                            out=pt[:, a:b], in_=stp[:, a:b],
                            func=mybir.ActivationFunctionType.Exp,
                            scale=0.125, bias=-SHIFT,
                        )
                return pt

            def emit_masks(entry):
                idx, chunk, pt, pms = entry
                h, j, k0, n_kt = chunk
                off0 = chunk_off(j, k0)
                if off0 < 0:
                    return
                if is_fp8(idx):
                    # zero masked fp8 bytes in place: AND with 0x00/0xFF
                    p = (k0 - 4 * j) // 2
                    nc.vector.tensor_tensor(
                        out=pt[:, :, off0:QB].bitcast(I16),
                        in0=pt[:, :, off0:QB].bitcast(I16),
                        in1=mk8s[:, p, :, off0 // 2:QB // 2],
                        op=mybir.AluOpType.bitwise_and,
                    )
                    return
                for r in range(2):
                    ki = k0 + r
                    t = ki - 4 * j
                    off = 128 * t
                    pm = pmpool.tile([128, QB], F16, name="pm", tag="pm")
                    nc.vector.tensor_mul(
                        out=pm[:, off:QB],
                        in0=pt[:, r * QB + off:(r + 1) * QB],
                        in1=dmasks[t][:, off:QB],
                    )
                    pms[r] = pm

            def emit_pvs(entry):
                idx, chunk, pt, pms = entry
                h, j, k0, n_kt = chunk
                vas, vas8, qkts, outs = head_ctx[h]
                off0 = max(chunk_off(j, k0), 0)
                if (h, j) not in otp_box:
                    otp_box[(h, j)] = otpool.tile(
                        [VW, QB], F32, name="otp", tag="otp"
                    )
                otp = otp_box[(h, j)]
                if is_fp8(idx):
                    nc.tensor.matmul(
                        out=otp[:, off0:QB],
                        lhsT=vas8[:, k0 // 2, :, 0:VW],
                        rhs=pt[:, :, off0:QB],
                        start=(k0 == 0),
                        stop=(k0 + 2 == n_kt),
                        perf_mode=mybir.MatmulPerfMode.DoubleRow,
                    )
                else:
                    for r in range(2):
                        ki = k0 + r
                        t = ki - 4 * j
                        off = 128 * t if t >= 0 else 0
                        if r in pms:
                            src = pms[r][:, off:QB]
                        else:
                            src = pt[:, r * QB + off:(r + 1) * QB]
                        nc.tensor.matmul(
                            out=otp[:, off:QB],
                            lhsT=vas[:, ki * VW:(ki + 1) * VW],
                            rhs=src,
                            start=(ki == 0),
                            stop=(ki == n_kt - 1),
                        )
                if k0 + 2 == n_kt:       # last chunk of this q-block
                    if copy_on_dve[idx]:
                        nc.vector.tensor_copy(
                            out=outs[:, j * QB:(j + 1) * QB], in_=otp
                        )
                    else:
                        nc.scalar.copy(
                            out=outs[:, j * QB:(j + 1) * QB], in_=otp
                        )
                    nc.sync.dma_start(
                        out=oT_d[h][:, j * QB:(j + 1) * QB],
                        in_=outs[:, j * QB:(j + 1) * QB],
                    )

            # 2-deep software pipeline.  Per-iteration emission order:
            #   VectorE: masks of chunk c-1 (before exp of chunk c)
            #   PE:      QK of chunk c ... PV of chunk c-2
            from collections import deque

            pend = deque()
            BATCH = 3
            for step in range(0, len(all_chunks), BATCH):
                grp = all_chunks[step:step + BATCH]
                # exps first: they gate the PSUM score-slot rotation (QK of
                # batch b+1 reuses slot of chunk c-3); masks of the previous
                # batch have ~2 batches of slack before their PV consumes them
                prev = list(pend)[-BATCH:]
                for off_i, chunk in enumerate(grp):
                    pt = emit_mm1s(step + off_i, chunk)
                    pend.append((step + off_i, chunk, pt, {}))
                for entry in prev:
                    emit_masks(entry)
                while len(pend) > 2 * BATCH:
                    emit_pvs(pend.popleft())
            for entry in list(pend)[-BATCH:]:
                emit_masks(entry)
            while pend:
                emit_pvs(pend.popleft())

    # TRN2 allows at most 1 semaphore wait per instruction; split surplus
    # waits into standalone EventSemaphore instructions like the bacc flow.
    import concourse.bacc as baccmod

    baccmod._bass_rust.generate_event_semaphores(nc)
    return nc


_PROGRAM_CACHE: dict[str, bass.Bass] = {}


def get_program() -> bass.Bass:
    key = "|".join(
        os.environ.get(k, "") for k in ("ATTN_WARM", "ATTN_DVE", "ATTN_FP8")
    )
    if key not in _PROGRAM_CACHE:
        _PROGRAM_CACHE[key] = build_program()
    return _PROGRAM_CACHE[key]


def make_masks() -> np.ndarray:
    kk = np.arange(128)[:, None]
    qq = np.arange(QB)[None, :]
    mk = np.empty((128, 4 * QB + 2), dtype=np.float16)
    for t in range(4):
        mk[:, t * QB:(t + 1) * QB] = (qq >= kk + 128 * t).astype(np.float16)
    mk[:, 4 * QB:4 * QB + 2] = (
        np.full((128, 1), -SHIFT, dtype=np.float32).view(np.float16)
    )
    return np.ascontiguousarray(mk)


def make_masks8() -> np.ndarray:
    kk = np.arange(128)[:, None]
    qq = np.arange(QB)[None, :]
    mk8 = np.empty((128, 2, 2, QB), dtype=np.uint8)
    for t in range(4):
        mk8[:, t // 2, t % 2, :] = np.where(qq >= kk + 128 * t, 0xFF, 0x00)
    return mk8.view(np.int16)


def make_in_maps(q, k, v):
    import ml_dtypes

    q = np.asarray(q, dtype=np.float32)
    k = np.asarray(k, dtype=np.float32)
    v = np.asarray(v, dtype=np.float32)
    mk = make_masks()
    mk8 = make_masks8()
    in_maps = []
    for c in range(N_CORES):
        hs = [H_PER * c + i for i in range(H_PER)]
        qk = np.empty((H_PER, 2 * D, 2 * S), dtype=np.float16)
        va = np.empty((H_PER, 128, NKT, VW), dtype=np.float16)
        va8 = np.zeros(
            (H_PER, 128, NKT // 2, 2, VWP), dtype=ml_dtypes.float8_e4m3
        )
        for i, h in enumerate(hs):
            qk[i, 0:D, 0:S] = q[0, h].T
            qk[i, 0:D, S:2 * S] = k[0, h].T
            qk[i, D:2 * D, :] = qk[i, 0:D, :]
            # [S, D] -> k-tiles on partitions: [128, NKT, D]
            vkt = v[0, h].reshape(NKT, KT, D).transpose(1, 0, 2)
            va[i, :, :, :D] = vkt
            va[i, :, :, D] = 1.0
            va8[i, :, :, :, :D] = vkt.reshape(128, NKT // 2, 2, D).astype(
                ml_dtypes.float8_e4m3
            )
            va8[i, :, :, :, D] = 1.0
        in_maps.append(
            {
                "qk": qk,
                "va": np.ascontiguousarray(va.reshape(H_PER, 128, NKT * VW)),
                "va8": va8,
                "mk": mk,
                "mk8": mk8,
            }
        )
    return in_maps


def assemble_output(results) -> np.ndarray:
    out = np.empty((B, H, S, D), dtype=np.float32)
    for c in range(N_CORES):
        oT = results[c]["outT"]  # [H_PER, VW, S]
        for i in range(H_PER):
            h = H_PER * c + i
            out[0, h] = (oT[i, :D, :] / oT[i, D:D + 1, :]).T
    return out


def run_sharded(q, k, v, trace: bool = False):
    from concourse.bass_utils import run_bass_kernel_spmd

    nc = get_program()
    in_maps = make_in_maps(q, k, v)
    res = run_bass_kernel_spmd(
        nc, in_maps, list(range(N_CORES)), trace=trace
    )
    return assemble_output(res.results), res


def kernel(q, k, v, mask=None) -> np.ndarray:
    # mask is deterministically the causal tril mask; causality is baked in.
    out, _ = run_sharded(q, k, v, trace=False)
    return out


# revision 36
# speedup vs baseline: 1.0170x; 1.0170x over previous
"""Causal attention (B=1, H=16, S=4096, D=64, f32) on 8 trn2 NeuronCores.

Strategy (head-parallel, 2 heads per core):
  - Host pre-transposes Q, K per head to [D, S] (d-major) so the QK^T
    matmul needs no on-device transpose: S^T[k, q] = sum_d K^T[d,k] Q^T[d,q].
  - S^T layout keeps k on PSUM partitions and q on the free axis, so
    exp(S^T) -> P^T lands in SBUF exactly as the lhsT of the PV matmul:
    O^T[d, q] = sum_k V[k, d] P^T[k, q], accumulated over k-tiles in PSUM.
  - p' = exp(s - 3.25): the global shift (softmax-invariant, cancels in
    the l division) keeps p' <= ~190 so it fits fp8e4m3.  l[q] = sum_k p'
    comes free from a ones column in V.
  - Causality: k-tiles strictly below the diagonal are skipped; diagonal
    k-tiles are masked post-exp (VectorE) and column-trimmed (for
    diagonal tile t only q >= 128t can be unmasked).
  - Host epilogue: O = (O^T_unnorm[:64] / l).T per head.

exp is split between ScalarE (native ACT exp) and VectorE (Schraudolph
bit-trick: p_bits = round(A*s + B) -> int16, bitcast fp16), load-balanced
at build time.  q-block 0 (rows with <512 keys, least error averaging)
is pinned to the exact ScalarE fp16 path.

PV matmul precision/speed:
  - ScalarE chunks (j>=1) emit p' in fp8e4m3; their PV runs as ONE
    DoubleRow matmul per 2 k-tiles (fp8 V, contraction 256 virtual rows,
    2 elem/cycle moving) - half the PE time of two fp16 matmuls.
    Diagonal masking for these is a bitwise AND (0x00/0xFF bytes) on the
    int16-bitcast fp8 pairs, in place.
  - VectorE chunks emit fp16 (int8 Schraudolph can't represent the fp8
    subnormal band correctly), PV is two regular fp16 matmuls.
  - q-block 0 is all-fp16 (fp8 V quantization is too coarse for rows
    attending few keys).

QK^T matmuls run fp16, two-at-a-time in disjoint PE row groups (rows
0-63 / 64-127 hold duplicate q,k data) - the trace confirms the second
of each pair retires in ~4ns.

Pipeline: chunks of 2 k-tiles, one 2-bank PSUM score tile each (3 bufs -
PSUM's 8 banks = 3x2 score slots + 2 output accumulators, the binding
resource).  Chunks are emitted in batches of 3 aligned to the score-slot
rotation: the 3 QK pairs run back-to-back (hiding each other's
LDWEIGHTS), PVs trail by 2 batches so they never head-of-line block the
PE queue, and each batch's exps are emitted before the previous batch's
masks (exps gate the score-slot rotation; masks have 2 batches of
slack).

The PE clock ramps from 1.2 to 2.4 GHz only after ~3.4us of sustained
matmul activity (HAM clock-gate); the first ~15 real matmuls run cold.
Warmup matmuls were tried and removed: 1-column const-sourced ones have
too little duty cycle to trip the gate, and higher-duty variants delay
the DMA-gated first matmul by more than they save.
"""

import os
import sys
import numpy as np

sys.path.insert(0, "/opt/trn_rl_repo")

import concourse.bass as bass
import concourse.mybir as mybir
from concourse.tile import TileContext

B, H, S, D = 1, 16, 4096, 64
N_CORES = 8
H_PER = H // N_CORES          # heads per core
QB = 512                      # q-block (matmul moving dim / PSUM bank)
KT = 128                      # k-tile (contraction tile for PV matmul)
NQB = S // QB                 # 8
NKT = S // KT                 # 32
VW = D + 1                    # V columns + ones column for the l sum
VWP = 80                      # fp8 V plane pitch (DoubleRow needs 16B-aligned)

F32 = mybir.dt.float32
F16 = mybir.dt.float16
F8 = mybir.dt.float8e4
I16 = mybir.dt.int16
BF16 = mybir.dt.bfloat16

LOG2E = 1.4426950408889634
SHIFT = 3.25                  # p' = exp(s - SHIFT); max p' ~ exp(8.44-3.25)=180
# Schraudolph exp for fp16 bit pattern: exp(0.125*s - SHIFT) ~=
# bitcast_fp16(round(A*s + B)); the -44.5 centers the relative error (~3%).
SCHRAU_A = 0.125 * LOG2E * 1024.0
SCHRAU_B = 15360.0 - 1024.0 * SHIFT * LOG2E - 44.5


def build_program() -> bass.Bass:
    dve_frac = float(os.environ.get("ATTN_DVE", "1"))
    use_fp8 = os.environ.get("ATTN_FP8", "1") != "0"
    n_warm = int(os.environ.get("ATTN_WARM", "0"))

    nc = bass.Bass()
    # qk rows 0-63 and 64-127 hold identical qT|kT data: the duplicate lets
    # two QK^T matmuls run concurrently in disjoint PE row groups
    qk_d = nc.declare_dram_parameter("qk", [H_PER, 2 * D, 2 * S], F16, isOutput=False)
    va_d = nc.declare_dram_parameter("va", [H_PER, 128, NKT * VW], F16, isOutput=False)
    va8_d = nc.declare_dram_parameter(
        "va8", [H_PER, 128, NKT // 2, 2, VWP], F8, isOutput=False
    )
    mk_d = nc.declare_dram_parameter("mk", [128, 4 * QB + 2], F16, isOutput=False)
    mk8_d = nc.declare_dram_parameter("mk8", [128, 2, 2, QB // 2], I16, isOutput=False)
    oT_d = nc.declare_dram_parameter("outT", [H_PER, VW, S], F32, isOutput=True)

    with TileContext(nc) as tc:
        with (
            tc.tile_pool(name="const", bufs=1) as cpool,
            tc.tile_pool(name="io", bufs=1) as iopool,
            tc.tile_pool(name="pt", bufs=8) as ppool,
            tc.tile_pool(name="pm", bufs=12) as pmpool,
            tc.tile_pool(name="st", bufs=3, space="PSUM") as stpool,
            tc.tile_pool(name="ot", bufs=2, space="PSUM") as otpool,
        ):
            # 0/1 masks for the 4 diagonal k-tiles of each q-block:
            # keep (1.0 / 0xFF) where qq >= kk + 128*t.
            mks = cpool.tile([128, 4 * QB + 2], F16, name="mks")
            nc.sync.dma_start(out=mks, in_=mk_d[:, :])
            dmasks = [mks[:, t * QB:(t + 1) * QB] for t in range(4)]
            # exp bias (-SHIFT) const AP: fp32 bit pattern embedded in the
            # last two fp16 mask columns (avoids a gpsimd memset + barrier)
            nc.const_aps.aps[(mybir.dt.float32, -SHIFT)] = (
                mks[:, 4 * QB:4 * QB + 2].bitcast(F32)
            )
            mk8s = cpool.tile([128, 2, 2, QB // 2], I16, name="mk8s")
            if use_fp8:
                nc.sync.dma_start(out=mk8s, in_=mk8_d[:, :, :, :])

            # Optional warmup matmuls (off by default: trace shows the
            # tiny 1-col matmuls have too little duty cycle to trip the PE
            # HAM clock-gate, so they were pure PE-queue time; the clock
            # ramps from real matmul activity either way).
            if n_warm:
                cb = nc.const_aps.aps[(mybir.dt.bfloat16, 1.0)]
                wps = otpool.tile([128, 16], F32, name="warmps", tag="otp")
                for _ in range(n_warm):
                    nc.tensor.matmul(
                        out=wps[0:1, 0:1], lhsT=cb, rhs=cb, start=True, stop=True,
                    )

            head_ctx = []
            for h in range(H_PER):
                vas = iopool.tile([128, NKT * VW], F16, name=f"vas{h}")
                vas8 = iopool.tile([128, NKT // 2, 2, VWP], F8, name=f"vas8{h}")
                qkts = iopool.tile([2 * D, 2 * S], F16, name=f"qkts{h}")
                outs = iopool.tile([VW, S], F32, name=f"outs{h}")
                # q-block 0 only needs the first 512 columns of q/k and the
                # first 4 V k-tiles: stage those first so compute starts
                # while the bulk still streams in
                if h == 0:
                    nc.sync.dma_start(out=vas[:, 0:4 * VW], in_=va_d[h][:, 0:4 * VW])
                    nc.sync.dma_start(out=qkts[:, 0:QB], in_=qk_d[h][:, 0:QB])
                    nc.sync.dma_start(
                        out=qkts[:, S:S + QB], in_=qk_d[h][:, S:S + QB]
                    )
                    nc.sync.dma_start(
                        out=vas[:, 4 * VW:], in_=va_d[h][:, 4 * VW:]
                    )
                    nc.sync.dma_start(out=qkts[:, QB:S], in_=qk_d[h][:, QB:S])
                    nc.sync.dma_start(
                        out=qkts[:, S + QB:2 * S], in_=qk_d[h][:, S + QB:2 * S]
                    )
                else:
                    nc.sync.dma_start(out=vas, in_=va_d[h])
                    # split halves onto separate DMA queues
                    nc.sync.dma_start(out=qkts[:, 0:S], in_=qk_d[h][:, 0:S])
                    nc.sync.dma_start(
                        out=qkts[:, S:2 * S], in_=qk_d[h][:, S:2 * S]
                    )
                if use_fp8:
                    nc.sync.dma_start(out=vas8, in_=va8_d[h])
                head_ctx.append((vas, vas8, qkts, outs))

            # flat chunk list over (head, q-block): 2 k-tiles per chunk.
            all_chunks = []
            for h in range(H_PER):
                for j in range(NQB):
                    n_kt = 4 * (j + 1)          # causal: k-tiles 0..4j+3
                    for k0 in range(0, n_kt, 2):
                        all_chunks.append((h, j, k0, n_kt))

            def chunk_off(j, k0):
                """Uniform column offset for the chunk (both k-tiles of a
                chunk are diagonal together); for diagonal pair (t, t+1)
                only q >= 128t can be unmasked."""
                t0 = k0 - 4 * j
                return 128 * t0 if t0 >= 0 else -1   # -1 = not diagonal

            # Build-time exp load balancing: ScalarE chunks (j>=1) go fp8
            # (DoubleRow PV); VectorE chunks go fp16 Schraudolph.  VectorE
            # also owns the diagonal masking.  q-block 0 stays exact fp16.
            exp_on_dve = {}
            copy_on_dve = {}
            load_s, load_d = 0.0, 0.0
            for idx, (h, j, k0, n_kt) in enumerate(all_chunks):
                off0 = chunk_off(j, k0)
                diag = off0 >= 0
                o = max(off0, 0)
                if diag:
                    t_s = 2 * ((QB - o) + 352) / 1.2
                    t_d = 2 * ((QB - o) + 151) / 0.96
                    # masks: fp8 chunk = one AND over both planes;
                    # fp16 chunk = one multiply per k-tile
                    m_s = ((QB - o) / 2 + 151) / 0.96 if use_fp8 else 2 * (
                        (QB - o) / 2 + 151
                    ) / 0.96
                    m_d = 2 * ((QB - o) / 2 + 151) / 0.96
                else:
                    t_s = (2 * QB + 352) / 1.2
                    t_d = (2 * QB + 151) / 0.96
                    m_s = m_d = 0.0
                if j == 0 or dve_frac == 0.0:
                    use_d = False
                else:
                    use_d = (load_d + (t_d + m_d) * dve_frac
                             < load_s + t_s + m_s - load_d * 0)
                exp_on_dve[idx] = use_d
                if use_d:
                    load_d += t_d + m_d
                else:
                    load_s += t_s
                    load_d += m_s
                if k0 + 2 == n_kt:   # q-block end: PSUM->SBUF copy
                    use_dc = load_d + 658 < load_s + 720
                    copy_on_dve[idx] = use_dc
                    if use_dc:
                        load_d += 658
                    else:
                        load_s += 720

            def is_fp8(idx):
                h, j, k0, n_kt = all_chunks[idx]
                return use_fp8 and j >= 1 and not exp_on_dve[idx]

            otp_box = {}

            def emit_mm1s(idx, chunk):
                h, j, k0, n_kt = chunk
                vas, vas8, qkts, outs = head_ctx[h]
                off0 = max(chunk_off(j, k0), 0)
                stp = stpool.tile([128, 2 * QB], F32, name="stp", tag="stp")
                for r in range(2):
                    ki = k0 + r
                    row = slice(r * D, (r + 1) * D)
                    nc.tensor.matmul(
                        out=stp[:, r * QB + off0:(r + 1) * QB],
                        lhsT=qkts[row, S + ki * KT:S + (ki + 1) * KT],
                        rhs=qkts[row, j * QB + off0:(j + 1) * QB],
                        start=True,
                        stop=True,
                    )
                if is_fp8(idx):
                    pt = ppool.tile([128, 2, QB], F8, name="pt8", tag="pt")
                    if off0 == 0:
                        nc.scalar.activation(
                            out=pt[:, :, :], in_=stp[:, 0:2 * QB],
                            func=mybir.ActivationFunctionType.Exp,
                            scale=0.125, bias=-SHIFT,
                        )
                    else:
                        for r in range(2):
                            nc.scalar.activation(
                                out=pt[:, r, off0:QB],
                                in_=stp[:, r * QB + off0:(r + 1) * QB],
                                func=mybir.ActivationFunctionType.Exp,
                                scale=0.125, bias=-SHIFT,
                            )
                    return pt
                pt = ppool.tile([128, 2 * QB], F16, name="pt", tag="pt")
                ranges = (
                    [(0, 2 * QB)] if off0 == 0
                    else [(r * QB + off0, (r + 1) * QB) for r in range(2)]
                )
                for a, b in ranges:
                    if exp_on_dve[idx]:
                        nc.vector.tensor_scalar(
                            out=pt[:, a:b].bitcast(I16),
                            in0=stp[:, a:b],
                            scalar1=SCHRAU_A,
                            scalar2=SCHRAU_B,
                            op0=mybir.AluOpType.mult,
                            op1=mybir.AluOpType.add,
                        )
                    else:
                        nc.scalar.activation(
                            out=pt[:, a:b], in_=stp[:, a:b],
                            func=mybir.ActivationFunctionType.Exp,
                            scale=0.125, bias=-SHIFT,
                        )
                return pt

            def emit_masks(entry):
                idx, chunk, pt, pms = entry
                h, j, k0, n_kt = chunk
                off0 = chunk_off(j, k0)
                if off0 < 0:
                    return
                if is_fp8(idx):
                    # zero masked fp8 bytes in place: AND with 0x00/0xFF
                    p = (k0 - 4 * j) // 2
                    nc.vector.tensor_tensor(
                        out=pt[:, :, off0:QB].bitcast(I16),
                        in0=pt[:, :, off0:QB].bitcast(I16),
                        in1=mk8s[:, p, :, off0 // 2:QB // 2],
                        op=mybir.AluOpType.bitwise_and,
                    )
                    return
                for r in range(2):
                    ki = k0 + r
                    t = ki - 4 * j
                    off = 128 * t
                    pm = pmpool.tile([128, QB], F16, name="pm", tag="pm")
                    nc.vector.tensor_mul(
                        out=pm[:, off:QB],
                        in0=pt[:, r * QB + off:(r + 1) * QB],
                        in1=dmasks[t][:, off:QB],
                    )
                    pms[r] = pm

            def emit_pvs(entry):
                idx, chunk, pt, pms = entry
                h, j, k0, n_kt = chunk
                vas, vas8, qkts, outs = head_ctx[h]
                off0 = max(chunk_off(j, k0), 0)
                if (h, j) not in otp_box:
                    otp_box[(h, j)] = otpool.tile(
                        [VW, QB], F32, name="otp", tag="otp"
                    )
                otp = otp_box[(h, j)]
                if is_fp8(idx):
                    nc.tensor.matmul(
                        out=otp[:, off0:QB],
                        lhsT=vas8[:, k0 // 2, :, 0:VW],
                        rhs=pt[:, :, off0:QB],
                        start=(k0 == 0),
                        stop=(k0 + 2 == n_kt),
                        perf_mode=mybir.MatmulPerfMode.DoubleRow,
                    )
                else:
                    for r in range(2):
                        ki = k0 + r
                        t = ki - 4 * j
                        off = 128 * t if t >= 0 else 0
                        if r in pms:
                            src = pms[r][:, off:QB]
                        else:
                            src = pt[:, r * QB + off:(r + 1) * QB]
                        nc.tensor.matmul(
                            out=otp[:, off:QB],
                            lhsT=vas[:, ki * VW:(ki + 1) * VW],
                            rhs=src,
                            start=(ki == 0),
                            stop=(ki == n_kt - 1),
                        )
                if k0 + 2 == n_kt:       # last chunk of this q-block
                    if copy_on_dve[idx]:
                        nc.vector.tensor_copy(
                            out=outs[:, j * QB:(j + 1) * QB], in_=otp
                        )
                    else:
                        nc.scalar.copy(
                            out=outs[:, j * QB:(j + 1) * QB], in_=otp
                        )
                    nc.sync.dma_start(
                        out=oT_d[h][:, j * QB:(j + 1) * QB],
                        in_=outs[:, j * QB:(j + 1) * QB],
                    )

            # 2-deep software pipeline.  Per-iteration emission order:
            #   VectorE: masks of chunk c-1 (before exp of chunk c)
            #   PE:      QK of chunk c ... PV of chunk c-2
            from collections import deque

            pend = deque()
            BATCH = 3
            for step in range(0, len(all_chunks), BATCH):
                grp = all_chunks[step:step + BATCH]
                # exps first: they gate the PSUM score-slot rotation (QK of
                # batch b+1 reuses slot of chunk c-3); masks of the previous
                # batch have ~2 batches of slack before their PV consumes them
                prev = list(pend)[-BATCH:]
                for off_i, chunk in enumerate(grp):
                    pt = emit_mm1s(step + off_i, chunk)
                    pend.append((step + off_i, chunk, pt, {}))
                for entry in prev:
                    emit_masks(entry)
                while len(pend) > 2 * BATCH:
                    emit_pvs(pend.popleft())
            for entry in list(pend)[-BATCH:]:
                emit_masks(entry)
            while pend:
                emit_pvs(pend.popleft())

    # TRN2 allows at most 1 semaphore wait per instruction; split surplus
    # waits into standalone EventSemaphore instructions like the bacc flow.
    import concourse.bacc as baccmod

    baccmod._bass_rust.generate_event_semaphores(nc)
    return nc


_PROGRAM_CACHE: dict[str, bass.Bass] = {}


def get_program() -> bass.Bass:
    key = "|".join(
        os.environ.get(k, "") for k in ("ATTN_WARM", "ATTN_DVE", "ATTN_FP8")
    )
    if key not in _PROGRAM_CACHE:
        _PROGRAM_CACHE[key] = build_program()
    return _PROGRAM_CACHE[key]


def make_masks() -> np.ndarray:
    kk = np.arange(128)[:, None]
    qq = np.arange(QB)[None, :]
    mk = np.empty((128, 4 * QB + 2), dtype=np.float16)
    for t in range(4):
        mk[:, t * QB:(t + 1) * QB] = (qq >= kk + 128 * t).astype(np.float16)
    mk[:, 4 * QB:4 * QB + 2] = (
        np.full((128, 1), -SHIFT, dtype=np.float32).view(np.float16)
    )
    return np.ascontiguousarray(mk)


def make_masks8() -> np.ndarray:
    kk = np.arange(128)[:, None]
    qq = np.arange(QB)[None, :]
    mk8 = np.empty((128, 2, 2, QB), dtype=np.uint8)
    for t in range(4):
        mk8[:, t // 2, t % 2, :] = np.where(qq >= kk + 128 * t, 0xFF, 0x00)
    return mk8.view(np.int16)


def make_in_maps(q, k, v):
    import ml_dtypes

    q = np.asarray(q, dtype=np.float32)
    k = np.asarray(k, dtype=np.float32)
    v = np.asarray(v, dtype=np.float32)
    mk = make_masks()
    mk8 = make_masks8()
    in_maps = []
    for c in range(N_CORES):
        hs = [H_PER * c + i for i in range(H_PER)]
        qk = np.empty((H_PER, 2 * D, 2 * S), dtype=np.float16)
        va = np.empty((H_PER, 128, NKT, VW), dtype=np.float16)
        va8 = np.zeros(
            (H_PER, 128, NKT // 2, 2, VWP), dtype=ml_dtypes.float8_e4m3
        )
        for i, h in enumerate(hs):
            qk[i, 0:D, 0:S] = q[0, h].T
            qk[i, 0:D, S:2 * S] = k[0, h].T
            qk[i, D:2 * D, :] = qk[i, 0:D, :]
            # [S, D] -> k-tiles on partitions: [128, NKT, D]
            vkt = v[0, h].reshape(NKT, KT, D).transpose(1, 0, 2)
            va[i, :, :, :D] = vkt
            va[i, :, :, D] = 1.0
            va8[i, :, :, :, :D] = vkt.reshape(128, NKT // 2, 2, D).astype(
                ml_dtypes.float8_e4m3
            )
            va8[i, :, :, :, D] = 1.0
        in_maps.append(
            {
                "qk": qk,
                "va": np.ascontiguousarray(va.reshape(H_PER, 128, NKT * VW)),
                "va8": va8,
                "mk": mk,
                "mk8": mk8,
            }
        )
    return in_maps


def assemble_output(results) -> np.ndarray:
    out = np.empty((B, H, S, D), dtype=np.float32)
    for c in range(N_CORES):
        oT = results[c]["outT"]  # [H_PER, VW, S]
        for i in range(H_PER):
            h = H_PER * c + i
            out[0, h] = (oT[i, :D, :] / oT[i, D:D + 1, :]).T
    return out


def run_sharded(q, k, v, trace: bool = False):
    from concourse.bass_utils import run_bass_kernel_spmd

    nc = get_program()
    in_maps = make_in_maps(q, k, v)
    res = run_bass_kernel_spmd(
        nc, in_maps, list(range(N_CORES)), trace=trace
    )
    return assemble_output(res.results), res


def kernel(q, k, v, mask=None) -> np.ndarray:
    # mask is deterministically the causal tril mask; causality is baked in.
    out, _ = run_sharded(q, k, v, trace=False)
    return out
